# revision 17
# baseline (speedup 1.0000x reference)
"""CRF-RNN layer (nn_CrfRnnLayer) as a Bass/Tile SPMD kernel on 8 TRN2 NeuronCores.

Algorithm (matches reference.py):
  N = 112*112 pixels, C = 21 classes, 5 mean-field iterations:
    sm = softmax(Q, axis=classes)
    spatial_out  = (sm @ Ks) / ns      Ks[i,j] = exp(-||p_i-p_j||^2 / (2*3^2))
    bilateral_out= (sm @ Kb) / nb      Kb from (pos/160, rgb/3) features
    Q = u - comp @ (sk @ spatial_out + bk @ bilateral_out)

Design (v2):
  - Pixel columns sharded 8 ways (1568 cols/core). Bilateral kernel slice
    E_b = Kb[:, cols] is computed once on-device (fp32r d2 matmul -> ACT exp)
    and stored *fully resident in SBUF as fp8-e4m3* (98 blocks of 128 pixels,
    ~150 KB/partition) -> zero DMA traffic in the main loop.
  - Main bilateral matmul: bf16 softmax lhsT (with a ones column at row 21
    whose PSUM row yields the normalizer nb) x fp8 E blocks = 1 cycle/row.
  - Q is AllGathered in bf16 pixel-major [N, C] layout, so the block-major
    softmax input reloads with a single strided DMA (no transposes).
  - Spatial filtering is separable with the 1/ns normalizers folded into the
    bf16 Gaussian matrices host-side. The softmaxed smB bounces through DRAM
    into [y,(x c)] layout; y-pass matmul, [k,(x c)]->[x,(k c)] bounce, x-pass
    matmul, then a small relayout into the stacked Q-update rhs. The whole
    chain overlaps the bilateral matmul.
  - SBUF pressure handled by tag-sharing temporally disjoint tiles.
"""

import numpy as np
import ml_dtypes

import concourse.mybir as mybir
import concourse.tile as tile
from concourse import bacc
from concourse.bass import _add_dep_helper
from concourse.bass_utils import run_bass_kernel_spmd

H = 112
W = 112
C = 21
N = H * W
NCORES = 8
YPC = H // NCORES            # 14 image rows per core
COLS = N // NCORES           # 1568 pixels per core
NB = 98                      # contraction blocks of 128 pixels
CTS = [(0, 512), (512, 512), (1024, 512), (1536, 32)]   # col tiles of 1568
NITER = 5
THETA_ALPHA = 160.0
THETA_BETA = 3.0
THETA_GAMMA = 3.0

F32 = mybir.dt.float32
F32R = mybir.dt.float32r
BF16 = mybir.dt.bfloat16
FP8 = mybir.dt.float8e4
EXPF = mybir.ActivationFunctionType.Exp

_CACHE = {}


def _build_program():
    nc = bacc.Bacc("TRN2", target_bir_lowering=False, debug=False, num_devices=NCORES)

    # Chain every PE matmul in emission order (ordering-only deps) so the
    # scheduler keeps same-weights matmuls adjacent for LDWEIGHTS dedup.
    _mm_state = {"prev": None}

    def mm(*args, **kwargs):
        inst = nc.tensor.matmul(*args, **kwargs)
        if _mm_state["prev"] is not None:
            _add_dep_helper(inst.ins, _mm_state["prev"].ins, sync=False,
                            reason="pe emission order")
        _mm_state["prev"] = inst
        return inst

    ub7 = nc.dram_tensor("ub7", [7, N], F32R, kind="ExternalInput")
    vb7 = nc.dram_tensor("vb7", [7, COLS], F32R, kind="ExternalInput")
    gxn = nc.dram_tensor("gxn", [W, W], BF16, kind="ExternalInput")
    gy2n = nc.dram_tensor("gy2n", [H, YPC], BF16, kind="ExternalInput")
    u21 = nc.dram_tensor("u21", [C, COLS], F32, kind="ExternalInput")
    qblk0 = nc.dram_tensor("qblk0", [128, NB * C], BF16, kind="ExternalInput")
    qy0 = nc.dram_tensor("qy0", [H, W * C], BF16, kind="ExternalInput")
    awT = nc.dram_tensor("awT", [54, C], F32, kind="ExternalInput")
    qt_out = nc.dram_tensor("qt_out", [C, COLS], F32, kind="ExternalOutput")

    with tile.TileContext(nc) as tc:
        with (
            tc.tile_pool(name="const", bufs=1) as cpool,
            tc.tile_pool(name="iter", bufs=1) as ipool,
            tc.tile_pool(name="stream", bufs=1) as stpool,
            tc.tile_pool(name="psum", bufs=1, space="PSUM") as pspool,
            tc.tile_pool(name="dram", bufs=1, space="DRAM") as dpool,
        ):
            # ---------------- constants ----------------
            gxn_sb = cpool.tile([W, W], BF16, tag="gxn", name="gxn")
            nc.sync.dma_start(gxn_sb[:], gxn[:])
            gy2n_sb = cpool.tile([H, YPC], BF16, tag="gy2n", name="gy2n")
            nc.sync.dma_start(gy2n_sb[:], gy2n[:])
            u21_sb = cpool.tile([C, COLS], F32, tag="u21", name="u21")
            nc.sync.dma_start(u21_sb[:], u21[:])
            awT_sb = cpool.tile([54, C], F32, tag="awT", name="awT")
            nc.sync.dma_start(awT_sb[:], awT[:])
            ones21 = cpool.tile([1, C], F32, tag="ones21", name="ones21")
            nc.gpsimd.memset(ones21[:], 1.0)
            invnb = cpool.tile([C, COLS], BF16, tag="invnb", name="invnb")

            # Q staged for the DVE 32x32 transpose: rows 21:32 stay zero.
            q32 = cpool.tile([32, COLS], BF16, tag="q32", name="q32")
            nc.gpsimd.memset(q32[:], 0.0)

            E_res = cpool.tile([128, NB, COLS], FP8, tag="eres", name="eres")

            qt_full = None

            def head(it):
                """Load Q, softmax block-major (bilateral lhsT) + y-major
                (spatial input). Returns (smB, smy)."""
                qblk = ipool.tile([128, NB * C], BF16, tag="qblk_smy",
                                  name=f"qblk_{it}")
                if it == 0:
                    nc.sync.dma_start(qblk[:], qblk0[:])
                else:
                    nc.sync.dma_start(
                        qblk[:].rearrange("r (b c) -> r b c", b=NB),
                        qt_full[:].rearrange("(r b) c -> r b c", r=128),
                    )
                eqB = ipool.tile([128, NB * C], BF16, tag="eqB_so54",
                                 name=f"eqB_{it}")
                nc.scalar.activation(eqB[:], qblk[:], EXPF)
                sums = ipool.tile([128, NB], F32, tag="sums", name=f"sums_{it}")
                nc.vector.reduce_sum(
                    sums[:], eqB[:].rearrange("p (b c) -> p b c", b=NB),
                    axis=mybir.AxisListType.X,
                )
                rsum = ipool.tile([128, NB], F32, tag="rsum", name=f"rsum_{it}")
                nc.vector.reciprocal(rsum[:], sums[:])
                smB = ipool.tile([128, NB, 33], BF16, tag="smB", name=f"smB_{it}")
                nc.gpsimd.memset(smB[:, :, C: 33], 1.0)
                nc.vector.tensor_mul(
                    smB[:, :, 0:C],
                    eqB[:].rearrange("p (b c) -> p b c", b=NB),
                    rsum[:].broadcast_to([128, NB, C]),
                )

                smy = ipool.tile([H, W * C], BF16, tag="qblk_smy",
                                 name=f"smy_{it}")
                if it == 0:
                    qy = ipool.tile([H, W * C], BF16, tag="qy_spc",
                                    name=f"qy_{it}")
                    nc.sync.dma_start(qy[:], qy0[:])
                    eqy = ipool.tile([H, W * C], BF16, tag="eqy_spT",
                                     name=f"eqy_{it}")
                    nc.scalar.activation(eqy[:], qy[:], EXPF)
                    sums2 = ipool.tile([H, W], F32, tag="sums", name=f"sums2_{it}")
                    nc.vector.reduce_sum(
                        sums2[:], eqy[:].rearrange("p (x c) -> p x c", x=W),
                        axis=mybir.AxisListType.X,
                    )
                    rsum2 = ipool.tile([H, W], F32, tag="rsum", name=f"rsum2_{it}")
                    nc.vector.reciprocal(rsum2[:], sums2[:])
                    nc.vector.tensor_mul(
                        smy[:].rearrange("p (x c) -> p x c", x=W),
                        eqy[:].rearrange("p (x c) -> p x c", x=W),
                        rsum2[:].broadcast_to([H, W, C]),
                    )
                else:
                    # compact copy -> both DMA sides fully contiguous
                    smc = ipool.tile([128, NB * C], BF16, tag="eqB_so54",
                                     name=f"smc_{it}")
                    nc.vector.tensor_copy(
                        smc[:].rearrange("p (b c) -> p b c", b=NB),
                        smB[:, :, 0:C],
                    )
                    sm_pm = dpool.tile([N, C], BF16, tag="smpm", bufs=2,
                                       name=f"smpm_{it}")
                    nc.sync.dma_start(
                        sm_pm[:].rearrange("(r b) (c) -> r (b c)", r=128), smc[:]
                    )
                    nc.sync.dma_start(
                        smy[:], sm_pm[:].rearrange("(y f) c -> y (f c)", y=H)
                    )
                return smB, smy

            # it0 head first: its ACT exp beats the 98 setup exps into the
            # strict-FIFO scalar queue, so iteration 0 starts right after
            # the d2 matmuls finish.
            head0 = head(0)

            # ---------------- setup: E_b = exp(d2) in fp8 ----------------
            vb7_sb = ipool.tile([7, COLS], F32R, tag="qT_qfin", name="vb7")
            nc.sync.dma_start(vb7_sb[:], vb7[:])
            for b in range(NB):
                if b % 2 == 0:
                    ubc = stpool.tile([7, 256], F32R, tag="ubc", bufs=2,
                                      name=f"ubc_{b}")
                    nc.sync.dma_start(
                        ubc[:, 0:min(256, (NB - b) * 128)],
                        ub7[:, b * 128: min(N, (b + 2) * 128)],
                    )
                ps = pspool.tile([128, 2048], F32, tag=("psA" if b % 2 == 0 else "psB"),
                                 name=f"d2_{b}")
                for ci, (c0, cw) in enumerate(CTS):
                    mm(ps[:, ci * 512: ci * 512 + cw],
                       ubc[:, (b % 2) * 128: (b % 2 + 1) * 128],
                       vb7_sb[:, c0: c0 + cw], start=True, stop=True)
                # bounce d2 out of PSUM via DVE (fast) so the ACT exp runs
                # off the PE/PSUM critical path
                eb = ipool.tile([128, COLS], BF16,
                                tag=("qy_spc" if b % 2 == 0 else "eqy_spT"),
                                name=f"eb_{b}")
                nc.vector.tensor_copy(eb[:], ps[:, 0:COLS])
                nc.scalar.activation(E_res[:, b, :], eb[:], EXPF)

            # ---------------- iterations ----------------
            for it in range(NITER):
                smB, smy = head0 if it == 0 else head(it)

                psB = pspool.tile([128, 2048], F32, tag="psB", name=f"psB_{it}")
                bl_ps = pspool.tile([33, 2048], F32, tag="psA", name=f"bl_{it}")

                def bl_chunk(b0, b1):
                    for b in range(b0, b1):
                        for ci, (c0, cw) in enumerate(CTS):
                            mm(bl_ps[:, ci * 512: ci * 512 + cw],
                               smB[:, b, :], E_res[:, b, c0: c0 + cw],
                               start=(b == 0), stop=(b == NB - 1))

                bl_chunk(0, 56)

                # y-pass: Tk[k, (x c)] = gy2n^T @ smy  (y'-normalizer folded)
                Tk = ipool.tile([YPC, W * C], BF16, tag="Tk", name=f"Tk_{it}")
                XCH = [(0, 24), (24, 24), (48, 24), (72, 24), (96, 16)]
                for k, (x0, xw) in enumerate(XCH):
                    mm(psB[0:YPC, (k % 2) * 512: (k % 2) * 512 + xw * C],
                       gy2n_sb[:], smy[:, x0 * C: (x0 + xw) * C],
                       start=True, stop=True)
                    nc.vector.tensor_copy(
                        Tk[:, x0 * C: (x0 + xw) * C],
                        psB[0:YPC, (k % 2) * 512: (k % 2) * 512 + xw * C],
                    )
                # bounce to x-on-partitions (33K elems only)
                td = dpool.tile([YPC, W * C], BF16, tag="td", bufs=2, name=f"td_{it}")
                nc.sync.dma_start(td[:], Tk[:])
                Txk = ipool.tile([W, YPC * C], BF16, tag="Txk", name=f"Txk_{it}")
                nc.sync.dma_start(
                    Txk[:].rearrange("x (k c) -> x k c", k=YPC),
                    td[:].rearrange("k (x c) -> x k c", x=W),
                )

                bl_chunk(56, 80)

                # x-pass in 4 chunks of 28 output columns; each [28, (k c)]
                # PSUM block is padded to [32, (k, 32)] and DVE-32-transposed
                # into class partitions, then copied into the stacked rhs.
                so54 = ipool.tile([54, COLS], F32, tag="eqB_so54",
                                  name=f"so54_{it}")
                for j, x0 in enumerate([0, 28, 56, 84]):
                    pslab = psB[0:28, 1024 + (j % 2) * 512:
                                 1024 + (j % 2) * 512 + YPC * C]
                    mm(pslab, gxn_sb[:, x0: x0 + 28], Txk[:],
                       start=True, stop=True)
                    spc = ipool.tile([32, YPC * 32], BF16, tag="qy_spc",
                                     name=f"spc_{it}_{j}")
                    nc.gpsimd.memset(spc[:], 0.0)
                    nc.vector.tensor_copy(
                        spc[:].rearrange("p (k c) -> p k c", c=32)[0:28, :, 0:C],
                        pslab.rearrange("p (k c) -> p k c", c=C),
                    )
                    spT = ipool.tile([32, YPC * 32], BF16, tag="eqy_spT",
                                     name=f"spT_{it}_{j}")
                    nc.vector.transpose(spT[:], spc[:])
                    nc.vector.tensor_copy(
                        so54[32: 32 + C, :]
                        .rearrange("c (k x) -> c k x", k=YPC)[:, :, x0: x0 + 28],
                        spT[:].rearrange("p (k x) -> p k x", x=32)[0:C, :, 0:28],
                    )

                bl_chunk(80, NB)

                # ---- iteration 0: invnb = 1/nb broadcast across class rows
                if it == 0:
                    rnb = ipool.tile([1, COLS], F32, tag="qT_qfin", name="rnb")
                    nc.vector.reciprocal(rnb[:], bl_ps[32:33, 0:COLS])
                    for ci, (c0, cw) in enumerate(CTS):
                        mm(psB[0:C, ci * 512: ci * 512 + cw],
                           ones21[:], rnb[:, c0: c0 + cw], start=True, stop=True)
                    nc.vector.tensor_copy(invnb[:], psB[0:C, 0:COLS])

                # ---- stacked rhs rows 0:21 bilateral (rows 21:32 zero)
                nc.gpsimd.memset(so54[0:32, :], 0.0)
                nc.vector.tensor_mul(so54[0:C, :], bl_ps[0:C, 0:COLS], invnb[:])

                # ---- Q = u + [A_b ; A_s]^T @ so54
                q_ps = pspool.tile([C, 2048], F32, tag="psA", name=f"qps_{it}")
                for ci, (c0, cw) in enumerate(CTS):
                    mm(q_ps[:, ci * 512: ci * 512 + cw],
                       awT_sb[:], so54[:, c0: c0 + cw], start=True, stop=True)

                if it < NITER - 1:
                    nc.vector.tensor_add(q32[0:C, :], q_ps[:, 0:COLS], u21_sb[:])
                    qT = ipool.tile([32, COLS], BF16, tag="qT_qfin", name=f"qT_{it}")
                    nc.vector.transpose(qT[:], q32[:])
                    qt_sl = dpool.tile([COLS, C], BF16, tag="qtsl", bufs=2,
                                       name=f"qtsl_{it}")
                    nc.sync.dma_start(
                        qt_sl[:].rearrange("(k p) c -> p k c", p=32),
                        qT[:].rearrange("p (k c) -> p k c", c=32)[:, :, 0:C],
                    )
                    qt_full = dpool.tile([N, C], BF16, tag="qtfull", bufs=2,
                                         addr_space="Shared", name=f"qtfull_{it}")
                    nc.gpsimd.collective_compute(
                        "AllGather",
                        mybir.AluOpType.bypass,
                        replica_groups=[list(range(NCORES))],
                        ins=[qt_sl[:]],
                        outs=[qt_full[:]],
                    )
                else:
                    q_fin = ipool.tile([C, COLS], F32, tag="qT_qfin", name="qfin")
                    nc.vector.tensor_add(q_fin[:], q_ps[:, 0:COLS], u21_sb[:])
                    nc.sync.dma_start(qt_out[:], q_fin[:])

    nc.compile()
    return nc


def _host_inputs(unaries, rgb, spatial_kernel, bilateral_kernel, compatibility_matrix):
    bf = ml_dtypes.bfloat16
    u = np.transpose(np.asarray(unaries, dtype=np.float32)[0], (2, 0, 1)).reshape(C, N)
    rgbf = np.asarray(rgb, dtype=np.float32)[0].reshape(N, 3)

    yy, xx = np.meshgrid(
        np.arange(H, dtype=np.float64), np.arange(W, dtype=np.float64), indexing="ij"
    )
    pos = np.stack([xx.ravel(), yy.ravel()], axis=1)  # [N, 2] (x, y)

    fb = np.concatenate(
        [pos / THETA_ALPHA, rgbf.astype(np.float64) / THETA_BETA], axis=1
    )
    fb -= fb.mean(axis=0, keepdims=True)  # centering: reduces fp32 cancellation
    fb32 = fb.astype(np.float32)
    sq = (fb32.astype(np.float64) ** 2).sum(axis=1)
    mhalf_sq = (-0.5 * sq).astype(np.float32)

    ub7_np = np.empty((7, N), np.float32)
    ub7_np[0:5] = fb32.T
    ub7_np[5] = mhalf_sq
    ub7_np[6] = 1.0
    vb7_np = np.empty((7, N), np.float32)
    vb7_np[0:5] = fb32.T
    vb7_np[5] = 1.0
    vb7_np[6] = mhalf_sq

    d = np.arange(-(H - 1), H, dtype=np.float64)
    g1tab = np.exp(-(d * d) / (2.0 * THETA_GAMMA**2))

    def g1(dd):
        return g1tab[np.asarray(dd) + (H - 1)]

    G = g1(np.arange(W)[:, None] - np.arange(W)[None, :])  # [t, t']
    s1 = np.array([g1(np.arange(H) - t).sum() for t in range(H)])
    gxn_np = np.ascontiguousarray((G / s1[None, :]).astype(bf))  # [x, x']

    comp = np.asarray(compatibility_matrix, dtype=np.float64)
    A_s = -(comp @ np.asarray(spatial_kernel, dtype=np.float64))
    A_b = -(comp @ np.asarray(bilateral_kernel, dtype=np.float64))
    awT_np = np.zeros((54, C), np.float32)
    awT_np[0:C] = A_b.T.astype(np.float32)
    awT_np[32: 32 + C] = A_s.T.astype(np.float32)

    uT = np.ascontiguousarray(u.T)  # [N, C]
    # contraction block b holds pixels {p*98+b}; DMA column (b,p) = pixel 98p+b
    X = (98 * np.arange(128)[None, :] + np.arange(NB)[:, None]).reshape(-1)
    ub7_np = np.ascontiguousarray(ub7_np[:, X])
    qblk0_np = np.ascontiguousarray(
        uT.reshape(128, NB, C).reshape(128, NB * C).astype(bf)
    )
    qy0_np = np.ascontiguousarray(uT.reshape(H, W * C).astype(bf))

    in_maps = []
    for c in range(NCORES):
        sl = slice(c * COLS, (c + 1) * COLS)
        dy = np.arange(H)[:, None] - (YPC * c + np.arange(YPC))[None, :]  # [y, k]
        gy2n_np = np.ascontiguousarray(
            (g1(dy) / s1[YPC * c + np.arange(YPC)][None, :]).astype(bf)
        )
        in_maps.append(
            dict(
                ub7=ub7_np,
                vb7=np.ascontiguousarray(vb7_np[:, sl]),
                gxn=gxn_np,
                gy2n=gy2n_np,
                u21=np.ascontiguousarray(u[:, sl]),
                qblk0=qblk0_np,
                qy0=qy0_np,
                awT=awT_np,
            )
        )
    return in_maps


def run(inputs, trace=False, **spmd_kwargs):
    in_maps = _host_inputs(**inputs)
    if "nc" not in _CACHE:
        _CACHE["nc"] = _build_program()
    nc = _CACHE["nc"]
    res = run_bass_kernel_spmd(
        nc, in_maps, core_ids=list(range(NCORES)), trace=trace, **spmd_kwargs
    )
    qs = [np.asarray(res.results[c]["qt_out"]) for c in range(NCORES)]
    Q = np.concatenate(qs, axis=1)  # [C, N]
    out = Q.reshape(C, H, W).transpose(1, 2, 0)[None].astype(np.float32)
    return out, res


def kernel(unaries, rgb, spatial_kernel, bilateral_kernel, compatibility_matrix):
    out, _ = run(
        dict(
            unaries=unaries,
            rgb=rgb,
            spatial_kernel=spatial_kernel,
            bilateral_kernel=bilateral_kernel,
            compatibility_matrix=compatibility_matrix,
        )
    )
    return out


# revision 19
# speedup vs baseline: 1.1808x; 1.1808x over previous
"""CRF-RNN layer (nn_CrfRnnLayer) as a Bass/Tile SPMD kernel on 8 TRN2 NeuronCores.

Algorithm (matches reference.py):
  N = 112*112 pixels, C = 21 classes, 5 mean-field iterations:
    sm = softmax(Q, axis=classes)
    spatial_out  = (sm @ Ks) / ns      Ks[i,j] = exp(-||p_i-p_j||^2 / (2*3^2))
    bilateral_out= (sm @ Kb) / nb      Kb from (pos/160, rgb/3) features
    Q = u - comp @ (sk @ spatial_out + bk @ bilateral_out)

Design (v2):
  - Pixel columns sharded 8 ways (1568 cols/core). Bilateral kernel slice
    E_b = Kb[:, cols] is computed once on-device (fp32r d2 matmul -> ACT exp)
    and stored *fully resident in SBUF as fp8-e4m3* (98 blocks of 128 pixels,
    ~150 KB/partition) -> zero DMA traffic in the main loop.
  - Main bilateral matmul: bf16 softmax lhsT (with a ones column at row 21
    whose PSUM row yields the normalizer nb) x fp8 E blocks = 1 cycle/row.
  - Q is AllGathered in bf16 pixel-major [N, C] layout, so the block-major
    softmax input reloads with a single strided DMA (no transposes).
  - Spatial filtering is separable with the 1/ns normalizers folded into the
    bf16 Gaussian matrices host-side. The softmaxed smB bounces through DRAM
    into [y,(x c)] layout; y-pass matmul, [k,(x c)]->[x,(k c)] bounce, x-pass
    matmul, then a small relayout into the stacked Q-update rhs. The whole
    chain overlaps the bilateral matmul.
  - SBUF pressure handled by tag-sharing temporally disjoint tiles.
"""

import numpy as np
import ml_dtypes

import concourse.mybir as mybir
import concourse.tile as tile
from concourse import bacc
from concourse.bass import _add_dep_helper
from concourse.bass_utils import run_bass_kernel_spmd

H = 112
W = 112
C = 21
N = H * W
NCORES = 8
YPC = H // NCORES            # 14 image rows per core
COLS = N // NCORES           # 1568 pixels per core
NB = 98                      # contraction blocks of 128 pixels
CTS = [(0, 512), (512, 512), (1024, 512), (1536, 32)]   # col tiles of 1568
NITER = 5
THETA_ALPHA = 160.0
THETA_BETA = 3.0
THETA_GAMMA = 3.0

F32 = mybir.dt.float32
F32R = mybir.dt.float32r
BF16 = mybir.dt.bfloat16
FP8 = mybir.dt.float8e4
EXPF = mybir.ActivationFunctionType.Exp

_CACHE = {}


def _build_program():
    nc = bacc.Bacc("TRN2", target_bir_lowering=False, debug=False, num_devices=NCORES)

    # Chain every PE matmul in emission order (ordering-only deps) so the
    # scheduler keeps same-weights matmuls adjacent for LDWEIGHTS dedup.
    _mm_state = {"prev": None}

    def mm(*args, **kwargs):
        inst = nc.tensor.matmul(*args, **kwargs)
        if _mm_state["prev"] is not None:
            _add_dep_helper(inst.ins, _mm_state["prev"].ins, sync=False,
                            reason="pe emission order")
        _mm_state["prev"] = inst
        return inst

    ub7 = nc.dram_tensor("ub7", [7, N], F32R, kind="ExternalInput")
    vb7 = nc.dram_tensor("vb7", [7, COLS], F32R, kind="ExternalInput")
    gxn = nc.dram_tensor("gxn", [W, W], BF16, kind="ExternalInput")
    gy2n = nc.dram_tensor("gy2n", [H, YPC], BF16, kind="ExternalInput")
    u21 = nc.dram_tensor("u21", [C, COLS], F32, kind="ExternalInput")
    qblk0 = nc.dram_tensor("qblk0", [128, NB * C], BF16, kind="ExternalInput")
    qy0 = nc.dram_tensor("qy0", [H, W * C], BF16, kind="ExternalInput")
    awT = nc.dram_tensor("awT", [54, C], F32, kind="ExternalInput")
    qt_out = nc.dram_tensor("qt_out", [C, COLS], F32, kind="ExternalOutput")

    with tile.TileContext(nc) as tc:
        with (
            tc.tile_pool(name="const", bufs=1) as cpool,
            tc.tile_pool(name="iter", bufs=1) as ipool,
            tc.tile_pool(name="stream", bufs=1) as stpool,
            tc.tile_pool(name="psum", bufs=1, space="PSUM") as pspool,
            tc.tile_pool(name="dram", bufs=1, space="DRAM") as dpool,
        ):
            # ---------------- constants ----------------
            gxn_sb = cpool.tile([W, W], BF16, tag="gxn", name="gxn")
            nc.sync.dma_start(gxn_sb[:], gxn[:])
            gy2n_sb = cpool.tile([H, YPC], BF16, tag="gy2n", name="gy2n")
            nc.sync.dma_start(gy2n_sb[:], gy2n[:])
            u21_sb = cpool.tile([C, COLS], F32, tag="u21", name="u21")
            nc.sync.dma_start(u21_sb[:], u21[:])
            awT_sb = cpool.tile([54, C], F32, tag="awT", name="awT")
            nc.sync.dma_start(awT_sb[:], awT[:])
            ones21 = cpool.tile([1, C], F32, tag="ones21", name="ones21")
            nc.gpsimd.memset(ones21[:], 1.0)
            invnb = cpool.tile([C, COLS], BF16, tag="invnb", name="invnb")

            # Q staged for the DVE 32x32 transpose: rows 21:32 stay zero.
            q32 = cpool.tile([32, COLS], BF16, tag="q32", name="q32")
            nc.gpsimd.memset(q32[:], 0.0)

            E_res = cpool.tile([128, NB, COLS], FP8, tag="eres", name="eres")

            qt_full = None

            def head(it):
                """Load Q, softmax block-major (bilateral lhsT) + y-major
                (spatial input). Returns (smB, smy)."""
                qblk = ipool.tile([128, NB * C], BF16, tag="qblk_smy",
                                  name=f"qblk_{it}")
                if it == 0:
                    nc.sync.dma_start(qblk[:], qblk0[:])
                else:
                    nc.sync.dma_start(
                        qblk[:].rearrange("r (b c) -> r b c", b=NB),
                        qt_full[:].rearrange("(r b) c -> r b c", r=128),
                    )
                eqB = ipool.tile([128, NB * C], BF16, tag="eqB_so54",
                                 name=f"eqB_{it}")
                nc.scalar.activation(eqB[:], qblk[:], EXPF)
                sums = ipool.tile([128, NB], F32, tag="sums", name=f"sums_{it}")
                nc.vector.reduce_sum(
                    sums[:], eqB[:].rearrange("p (b c) -> p b c", b=NB),
                    axis=mybir.AxisListType.X,
                )
                rsum = ipool.tile([128, NB], F32, tag="rsum", name=f"rsum_{it}")
                nc.vector.reciprocal(rsum[:], sums[:])
                smB = ipool.tile([128, NB, 32], FP8, tag="smB", name=f"smB_{it}")
                nc.gpsimd.memset(smB[:, :, C: C + 1], 1.0)
                nc.gpsimd.memset(smB[:, :, C + 1: 32], 0.0)
                nc.vector.tensor_mul(
                    smB[:, :, 0:C],
                    eqB[:].rearrange("p (b c) -> p b c", b=NB),
                    rsum[:].broadcast_to([128, NB, C]),
                )

                smy = ipool.tile([H, W * C], BF16, tag="qblk_smy",
                                 name=f"smy_{it}")
                if it == 0:
                    qy = ipool.tile([H, W * C], BF16, tag="qy_spc",
                                    name=f"qy_{it}")
                    nc.sync.dma_start(qy[:], qy0[:])
                    eqy = ipool.tile([H, W * C], BF16, tag="eqy_spT",
                                     name=f"eqy_{it}")
                    nc.scalar.activation(eqy[:], qy[:], EXPF)
                    sums2 = ipool.tile([H, W], F32, tag="sums", name=f"sums2_{it}")
                    nc.vector.reduce_sum(
                        sums2[:], eqy[:].rearrange("p (x c) -> p x c", x=W),
                        axis=mybir.AxisListType.X,
                    )
                    rsum2 = ipool.tile([H, W], F32, tag="rsum", name=f"rsum2_{it}")
                    nc.vector.reciprocal(rsum2[:], sums2[:])
                    nc.vector.tensor_mul(
                        smy[:].rearrange("p (x c) -> p x c", x=W),
                        eqy[:].rearrange("p (x c) -> p x c", x=W),
                        rsum2[:].broadcast_to([H, W, C]),
                    )
                else:
                    # compact copy -> both DMA sides fully contiguous
                    smc = ipool.tile([128, NB * C], BF16, tag="eqB_so54",
                                     name=f"smc_{it}")
                    nc.vector.tensor_copy(
                        smc[:].rearrange("p (b c) -> p b c", b=NB),
                        smB[:, :, 0:C],
                    )
                    sm_pm = dpool.tile([N, C], BF16, tag="smpm", bufs=2,
                                       name=f"smpm_{it}")
                    nc.sync.dma_start(
                        sm_pm[:].rearrange("(r b) (c) -> r (b c)", r=128), smc[:]
                    )
                    nc.sync.dma_start(
                        smy[:], sm_pm[:].rearrange("(y f) c -> y (f c)", y=H)
                    )
                return smB, smy

            # it0 head first: its ACT exp beats the 98 setup exps into the
            # strict-FIFO scalar queue, so iteration 0 starts right after
            # the d2 matmuls finish.
            head0 = head(0)

            # ---------------- setup: E_b = exp(d2) in fp8 ----------------
            vb7_sb = ipool.tile([7, COLS], F32R, tag="qT_qfin", name="vb7")
            nc.sync.dma_start(vb7_sb[:], vb7[:])
            for b in range(NB):
                if b % 2 == 0:
                    ubc = stpool.tile([7, 256], F32R, tag="ubc", bufs=2,
                                      name=f"ubc_{b}")
                    nc.sync.dma_start(
                        ubc[:, 0:min(256, (NB - b) * 128)],
                        ub7[:, b * 128: min(N, (b + 2) * 128)],
                    )
                ps = pspool.tile([128, 2048], F32, tag=("psA" if b % 2 == 0 else "psB"),
                                 name=f"d2_{b}")
                for ci, (c0, cw) in enumerate(CTS):
                    mm(ps[:, ci * 512: ci * 512 + cw],
                       ubc[:, (b % 2) * 128: (b % 2 + 1) * 128],
                       vb7_sb[:, c0: c0 + cw], start=True, stop=True)
                nc.scalar.activation(E_res[:, b, :], ps[:, 0:COLS], EXPF)

            # ---------------- iterations ----------------
            for it in range(NITER):
                smB, smy = head0 if it == 0 else head(it)

                psB = pspool.tile([128, 2048], F32, tag="psB", name=f"psB_{it}")
                bl_ps = pspool.tile([32, 2048], F32, tag="psA", name=f"bl_{it}")

                def bl_chunk(b0, b1):
                    for b in range(b0, b1, 2):
                        for ci, (c0, cw) in enumerate(CTS):
                            mm(bl_ps[:, ci * 512: ci * 512 + cw],
                               smB[:, b: b + 2, :], E_res[:, b: b + 2, c0: c0 + cw],
                               start=(b == 0), stop=(b == NB - 2),
                               perf_mode=mybir.MatmulPerfMode.DoubleRow)

                bl_chunk(0, 56)

                # y-pass: Tk[k, (x c)] = gy2n^T @ smy  (y'-normalizer folded)
                Tk = ipool.tile([YPC, W * C], BF16, tag="Tk", name=f"Tk_{it}")
                XCH = [(0, 24), (24, 24), (48, 24), (72, 24), (96, 16)]
                for k, (x0, xw) in enumerate(XCH):
                    mm(psB[0:YPC, (k % 2) * 512: (k % 2) * 512 + xw * C],
                       gy2n_sb[:], smy[:, x0 * C: (x0 + xw) * C],
                       start=True, stop=True)
                    nc.vector.tensor_copy(
                        Tk[:, x0 * C: (x0 + xw) * C],
                        psB[0:YPC, (k % 2) * 512: (k % 2) * 512 + xw * C],
                    )
                # bounce to x-on-partitions (33K elems only)
                td = dpool.tile([YPC, W * C], BF16, tag="td", bufs=2, name=f"td_{it}")
                nc.sync.dma_start(td[:], Tk[:])
                Txk = ipool.tile([W, YPC * C], BF16, tag="Txk", name=f"Txk_{it}")
                nc.sync.dma_start(
                    Txk[:].rearrange("x (k c) -> x k c", k=YPC),
                    td[:].rearrange("k (x c) -> x k c", x=W),
                )

                bl_chunk(56, 80)

                # x-pass in 4 chunks of 28 output columns; each [28, (k c)]
                # PSUM block is padded to [32, (k, 32)] and DVE-32-transposed
                # into class partitions, then copied into the stacked rhs.
                so54 = ipool.tile([54, COLS], F32, tag="eqB_so54",
                                  name=f"so54_{it}")
                for j, x0 in enumerate([0, 28, 56, 84]):
                    pslab = psB[0:28, 1024 + (j % 2) * 512:
                                 1024 + (j % 2) * 512 + YPC * C]
                    mm(pslab, gxn_sb[:, x0: x0 + 28], Txk[:],
                       start=True, stop=True)
                    spc = ipool.tile([32, YPC * 32], BF16, tag="qy_spc",
                                     name=f"spc_{it}_{j}")
                    nc.gpsimd.memset(spc[:], 0.0)
                    nc.vector.tensor_copy(
                        spc[:].rearrange("p (k c) -> p k c", c=32)[0:28, :, 0:C],
                        pslab.rearrange("p (k c) -> p k c", c=C),
                    )
                    spT = ipool.tile([32, YPC * 32], BF16, tag="eqy_spT",
                                     name=f"spT_{it}_{j}")
                    nc.vector.transpose(spT[:], spc[:])
                    nc.vector.tensor_copy(
                        so54[32: 32 + C, :]
                        .rearrange("c (k x) -> c k x", k=YPC)[:, :, x0: x0 + 28],
                        spT[:].rearrange("p (k x) -> p k x", x=32)[0:C, :, 0:28],
                    )

                bl_chunk(80, NB)

                # ---- iteration 0: invnb = 1/nb broadcast across class rows
                if it == 0:
                    rnb32 = ipool.tile([32, COLS], F32, tag="qT_qfin", name="rnb32")
                    nc.vector.reciprocal(rnb32[:], bl_ps[:, 0:COLS])
                    rnb_d = dpool.tile([1, COLS], F32, tag="rnbd", name="rnbd")
                    nc.sync.dma_start(rnb_d[:], rnb32[C: C + 1, :])
                    nc.sync.dma_start(rnb32[0:1, :], rnb_d[:])
                    for ci, (c0, cw) in enumerate(CTS):
                        mm(psB[0:C, ci * 512: ci * 512 + cw],
                           ones21[:], rnb32[0:1, c0: c0 + cw], start=True, stop=True)
                    nc.vector.tensor_copy(invnb[:], psB[0:C, 0:COLS])

                # ---- stacked rhs rows 0:21 bilateral (rows 21:32 zero)
                nc.gpsimd.memset(so54[0:32, :], 0.0)
                nc.vector.tensor_mul(so54[0:C, :], bl_ps[0:C, 0:COLS], invnb[:])

                # ---- Q = u + [A_b ; A_s]^T @ so54
                q_ps = pspool.tile([C, 2048], F32, tag="psA", name=f"qps_{it}")
                for ci, (c0, cw) in enumerate(CTS):
                    mm(q_ps[:, ci * 512: ci * 512 + cw],
                       awT_sb[:], so54[:, c0: c0 + cw], start=True, stop=True)

                if it < NITER - 1:
                    nc.vector.tensor_add(q32[0:C, :], q_ps[:, 0:COLS], u21_sb[:])
                    qT = ipool.tile([32, COLS], BF16, tag="qT_qfin", name=f"qT_{it}")
                    nc.vector.transpose(qT[:], q32[:])
                    qt_sl = dpool.tile([COLS, C], BF16, tag="qtsl", bufs=2,
                                       name=f"qtsl_{it}")
                    nc.sync.dma_start(
                        qt_sl[:].rearrange("(k p) c -> p k c", p=32),
                        qT[:].rearrange("p (k c) -> p k c", c=32)[:, :, 0:C],
                    )
                    qt_full = dpool.tile([N, C], BF16, tag="qtfull", bufs=2,
                                         addr_space="Shared", name=f"qtfull_{it}")
                    nc.gpsimd.collective_compute(
                        "AllGather",
                        mybir.AluOpType.bypass,
                        replica_groups=[list(range(NCORES))],
                        ins=[qt_sl[:]],
                        outs=[qt_full[:]],
                    )
                else:
                    q_fin = ipool.tile([C, COLS], F32, tag="qT_qfin", name="qfin")
                    nc.vector.tensor_add(q_fin[:], q_ps[:, 0:COLS], u21_sb[:])
                    nc.sync.dma_start(qt_out[:], q_fin[:])

    nc.compile()
    return nc


def _host_inputs(unaries, rgb, spatial_kernel, bilateral_kernel, compatibility_matrix):
    bf = ml_dtypes.bfloat16
    u = np.transpose(np.asarray(unaries, dtype=np.float32)[0], (2, 0, 1)).reshape(C, N)
    rgbf = np.asarray(rgb, dtype=np.float32)[0].reshape(N, 3)

    yy, xx = np.meshgrid(
        np.arange(H, dtype=np.float64), np.arange(W, dtype=np.float64), indexing="ij"
    )
    pos = np.stack([xx.ravel(), yy.ravel()], axis=1)  # [N, 2] (x, y)

    fb = np.concatenate(
        [pos / THETA_ALPHA, rgbf.astype(np.float64) / THETA_BETA], axis=1
    )
    fb -= fb.mean(axis=0, keepdims=True)  # centering: reduces fp32 cancellation
    fb32 = fb.astype(np.float32)
    sq = (fb32.astype(np.float64) ** 2).sum(axis=1)
    mhalf_sq = (-0.5 * sq).astype(np.float32)

    ub7_np = np.empty((7, N), np.float32)
    ub7_np[0:5] = fb32.T
    ub7_np[5] = mhalf_sq
    ub7_np[6] = 1.0
    vb7_np = np.empty((7, N), np.float32)
    vb7_np[0:5] = fb32.T
    vb7_np[5] = 1.0
    vb7_np[6] = mhalf_sq

    d = np.arange(-(H - 1), H, dtype=np.float64)
    g1tab = np.exp(-(d * d) / (2.0 * THETA_GAMMA**2))

    def g1(dd):
        return g1tab[np.asarray(dd) + (H - 1)]

    G = g1(np.arange(W)[:, None] - np.arange(W)[None, :])  # [t, t']
    s1 = np.array([g1(np.arange(H) - t).sum() for t in range(H)])
    gxn_np = np.ascontiguousarray((G / s1[None, :]).astype(bf))  # [x, x']

    comp = np.asarray(compatibility_matrix, dtype=np.float64)
    A_s = -(comp @ np.asarray(spatial_kernel, dtype=np.float64))
    A_b = -(comp @ np.asarray(bilateral_kernel, dtype=np.float64))
    awT_np = np.zeros((54, C), np.float32)
    awT_np[0:C] = A_b.T.astype(np.float32)
    awT_np[32: 32 + C] = A_s.T.astype(np.float32)

    uT = np.ascontiguousarray(u.T)  # [N, C]
    # contraction block b holds pixels {p*98+b}; DMA column (b,p) = pixel 98p+b
    X = (98 * np.arange(128)[None, :] + np.arange(NB)[:, None]).reshape(-1)
    ub7_np = np.ascontiguousarray(ub7_np[:, X])
    qblk0_np = np.ascontiguousarray(
        uT.reshape(128, NB, C).reshape(128, NB * C).astype(bf)
    )
    qy0_np = np.ascontiguousarray(uT.reshape(H, W * C).astype(bf))

    in_maps = []
    for c in range(NCORES):
        sl = slice(c * COLS, (c + 1) * COLS)
        dy = np.arange(H)[:, None] - (YPC * c + np.arange(YPC))[None, :]  # [y, k]
        gy2n_np = np.ascontiguousarray(
            (g1(dy) / s1[YPC * c + np.arange(YPC)][None, :]).astype(bf)
        )
        in_maps.append(
            dict(
                ub7=ub7_np,
                vb7=np.ascontiguousarray(vb7_np[:, sl]),
                gxn=gxn_np,
                gy2n=gy2n_np,
                u21=np.ascontiguousarray(u[:, sl]),
                qblk0=qblk0_np,
                qy0=qy0_np,
                awT=awT_np,
            )
        )
    return in_maps


def run(inputs, trace=False, **spmd_kwargs):
    in_maps = _host_inputs(**inputs)
    if "nc" not in _CACHE:
        _CACHE["nc"] = _build_program()
    nc = _CACHE["nc"]
    res = run_bass_kernel_spmd(
        nc, in_maps, core_ids=list(range(NCORES)), trace=trace, **spmd_kwargs
    )
    qs = [np.asarray(res.results[c]["qt_out"]) for c in range(NCORES)]
    Q = np.concatenate(qs, axis=1)  # [C, N]
    out = Q.reshape(C, H, W).transpose(1, 2, 0)[None].astype(np.float32)
    return out, res


def kernel(unaries, rgb, spatial_kernel, bilateral_kernel, compatibility_matrix):
    out, _ = run(
        dict(
            unaries=unaries,
            rgb=rgb,
            spatial_kernel=spatial_kernel,
            bilateral_kernel=bilateral_kernel,
            compatibility_matrix=compatibility_matrix,
        )
    )
    return out


# revision 20
# speedup vs baseline: 1.2572x; 1.0647x over previous
"""CRF-RNN layer (nn_CrfRnnLayer) as a Bass/Tile SPMD kernel on 8 TRN2 NeuronCores.

Algorithm (matches reference.py):
  N = 112*112 pixels, C = 21 classes, 5 mean-field iterations:
    sm = softmax(Q, axis=classes)
    spatial_out  = (sm @ Ks) / ns      Ks[i,j] = exp(-||p_i-p_j||^2 / (2*3^2))
    bilateral_out= (sm @ Kb) / nb      Kb from (pos/160, rgb/3) features
    Q = u - comp @ (sk @ spatial_out + bk @ bilateral_out)

Design (v2):
  - Pixel columns sharded 8 ways (1568 cols/core). Bilateral kernel slice
    E_b = Kb[:, cols] is computed once on-device (fp32r d2 matmul -> ACT exp)
    and stored *fully resident in SBUF as fp8-e4m3* (98 blocks of 128 pixels,
    ~150 KB/partition) -> zero DMA traffic in the main loop.
  - Main bilateral matmul: bf16 softmax lhsT (with a ones column at row 21
    whose PSUM row yields the normalizer nb) x fp8 E blocks = 1 cycle/row.
  - Q is AllGathered in bf16 pixel-major [N, C] layout, so the block-major
    softmax input reloads with a single strided DMA (no transposes).
  - Spatial filtering is separable with the 1/ns normalizers folded into the
    bf16 Gaussian matrices host-side. The softmaxed smB bounces through DRAM
    into [y,(x c)] layout; y-pass matmul, [k,(x c)]->[x,(k c)] bounce, x-pass
    matmul, then a small relayout into the stacked Q-update rhs. The whole
    chain overlaps the bilateral matmul.
  - SBUF pressure handled by tag-sharing temporally disjoint tiles.
"""

import numpy as np
import ml_dtypes

import concourse.mybir as mybir
import concourse.tile as tile
from concourse import bacc
from concourse.bass import _add_dep_helper
from concourse.bass_utils import run_bass_kernel_spmd

H = 112
W = 112
C = 21
N = H * W
NCORES = 8
YPC = H // NCORES            # 14 image rows per core
COLS = N // NCORES           # 1568 pixels per core
NB = 98                      # contraction blocks of 128 pixels
CTS = [(0, 512), (512, 512), (1024, 512), (1536, 32)]   # col tiles of 1568
NITER = 5
THETA_ALPHA = 160.0
THETA_BETA = 3.0
THETA_GAMMA = 3.0

F32 = mybir.dt.float32
F32R = mybir.dt.float32r
BF16 = mybir.dt.bfloat16
FP8 = mybir.dt.float8e4
EXPF = mybir.ActivationFunctionType.Exp

_CACHE = {}


def _build_program():
    nc = bacc.Bacc("TRN2", target_bir_lowering=False, debug=False, num_devices=NCORES)

    # Chain every PE matmul in emission order (ordering-only deps) so the
    # scheduler keeps same-weights matmuls adjacent for LDWEIGHTS dedup.
    _mm_state = {"prev": None}

    def mm(*args, **kwargs):
        inst = nc.tensor.matmul(*args, **kwargs)
        if _mm_state["prev"] is not None:
            _add_dep_helper(inst.ins, _mm_state["prev"].ins, sync=False,
                            reason="pe emission order")
        _mm_state["prev"] = inst
        return inst

    ub7 = nc.dram_tensor("ub7", [7, N], F32R, kind="ExternalInput")
    vb7 = nc.dram_tensor("vb7", [7, COLS], F32R, kind="ExternalInput")
    gxn = nc.dram_tensor("gxn", [W, W], BF16, kind="ExternalInput")
    gy2n = nc.dram_tensor("gy2n", [H, YPC], BF16, kind="ExternalInput")
    u21 = nc.dram_tensor("u21", [C, COLS], F32, kind="ExternalInput")
    qblk0 = nc.dram_tensor("qblk0", [128, NB * C], BF16, kind="ExternalInput")
    qy0 = nc.dram_tensor("qy0", [H, W * C], BF16, kind="ExternalInput")
    awT = nc.dram_tensor("awT", [54, C], F32, kind="ExternalInput")
    qt_out = nc.dram_tensor("qt_out", [C, COLS], F32, kind="ExternalOutput")

    with tile.TileContext(nc) as tc:
        with (
            tc.tile_pool(name="const", bufs=1) as cpool,
            tc.tile_pool(name="iter", bufs=1) as ipool,
            tc.tile_pool(name="stream", bufs=1) as stpool,
            tc.tile_pool(name="psum", bufs=1, space="PSUM") as pspool,
            tc.tile_pool(name="dram", bufs=1, space="DRAM") as dpool,
        ):
            # ---------------- warmup collective ----------------
            # absorbs ncfw first-call latency + cross-core skew while the
            # PE/ACT chew through setup; TOPSP/SDMA only, engines stay free
            wsrc = cpool.tile([1, 8], BF16, tag="warm", name="warm")
            nc.gpsimd.memset(wsrc[:], 0.0)
            warm_sl = dpool.tile([8, 1], BF16, tag="warmsl", name="warmsl")
            nc.sync.dma_start(warm_sl[:], wsrc[:].rearrange("p f -> f p"))
            warm_full = dpool.tile([64, 1], BF16, tag="warmfull",
                                   addr_space="Shared", name="warmfull")
            nc.gpsimd.collective_compute(
                "AllGather", mybir.AluOpType.bypass,
                replica_groups=[list(range(NCORES))],
                ins=[warm_sl[:]], outs=[warm_full[:]],
            )

            # ---------------- constants ----------------
            gxn_sb = cpool.tile([W, W], BF16, tag="gxn", name="gxn")
            nc.sync.dma_start(gxn_sb[:], gxn[:])
            gy2n_sb = cpool.tile([H, YPC], BF16, tag="gy2n", name="gy2n")
            nc.sync.dma_start(gy2n_sb[:], gy2n[:])
            u21_sb = cpool.tile([C, COLS], F32, tag="u21", name="u21")
            nc.sync.dma_start(u21_sb[:], u21[:])
            awT_sb = cpool.tile([54, C], F32, tag="awT", name="awT")
            nc.sync.dma_start(awT_sb[:], awT[:])
            ones21 = cpool.tile([1, C], F32, tag="ones21", name="ones21")
            nc.gpsimd.memset(ones21[:], 1.0)
            invnb = cpool.tile([C, COLS], BF16, tag="invnb", name="invnb")

            # Q staged for the DVE 32x32 transpose: rows 21:32 stay zero.
            q32 = cpool.tile([32, COLS], BF16, tag="q32", name="q32")
            nc.gpsimd.memset(q32[:], 0.0)

            E_res = cpool.tile([128, NB, COLS], FP8, tag="eres", name="eres")

            qt_full = None

            def head(it):
                """Load Q, softmax block-major (bilateral lhsT) + y-major
                (spatial input). Returns (smB, smy)."""
                qblk = ipool.tile([128, NB * C], BF16, tag="qblk_smy",
                                  name=f"qblk_{it}")
                if it == 0:
                    nc.sync.dma_start(qblk[:], qblk0[:])
                else:
                    nc.sync.dma_start(
                        qblk[:].rearrange("r (b c) -> r b c", b=NB),
                        qt_full[:].rearrange("(r b) c -> r b c", r=128),
                    )
                eqB = ipool.tile([128, NB * C], BF16, tag="eqB_so54",
                                 name=f"eqB_{it}")
                nc.scalar.activation(eqB[:], qblk[:], EXPF)
                sums = ipool.tile([128, NB], F32, tag="sums", name=f"sums_{it}")
                nc.vector.reduce_sum(
                    sums[:], eqB[:].rearrange("p (b c) -> p b c", b=NB),
                    axis=mybir.AxisListType.X,
                )
                rsum = ipool.tile([128, NB], F32, tag="rsum", name=f"rsum_{it}")
                nc.vector.reciprocal(rsum[:], sums[:])
                smB = ipool.tile([128, NB, 32], FP8, tag="smB", name=f"smB_{it}")
                nc.gpsimd.memset(smB[:, :, C: C + 1], 1.0)
                nc.gpsimd.memset(smB[:, :, C + 1: 32], 0.0)
                nc.vector.tensor_mul(
                    smB[:, :, 0:C],
                    eqB[:].rearrange("p (b c) -> p b c", b=NB),
                    rsum[:].broadcast_to([128, NB, C]),
                )

                smy = ipool.tile([H, W * C], BF16, tag="qblk_smy",
                                 name=f"smy_{it}")
                if it == 0:
                    qy = ipool.tile([H, W * C], BF16, tag="qy_spc",
                                    name=f"qy_{it}")
                    nc.sync.dma_start(qy[:], qy0[:])
                    eqy = ipool.tile([H, W * C], BF16, tag="eqy_spT",
                                     name=f"eqy_{it}")
                    nc.scalar.activation(eqy[:], qy[:], EXPF)
                    sums2 = ipool.tile([H, W], F32, tag="sums", name=f"sums2_{it}")
                    nc.vector.reduce_sum(
                        sums2[:], eqy[:].rearrange("p (x c) -> p x c", x=W),
                        axis=mybir.AxisListType.X,
                    )
                    rsum2 = ipool.tile([H, W], F32, tag="rsum", name=f"rsum2_{it}")
                    nc.vector.reciprocal(rsum2[:], sums2[:])
                    nc.vector.tensor_mul(
                        smy[:].rearrange("p (x c) -> p x c", x=W),
                        eqy[:].rearrange("p (x c) -> p x c", x=W),
                        rsum2[:].broadcast_to([H, W, C]),
                    )
                else:
                    # compact copy -> both DMA sides fully contiguous
                    smc = ipool.tile([128, NB * C], BF16, tag="eqB_so54",
                                     name=f"smc_{it}")
                    nc.vector.tensor_copy(
                        smc[:].rearrange("p (b c) -> p b c", b=NB),
                        smB[:, :, 0:C],
                    )
                    sm_pm = dpool.tile([N, C], BF16, tag="smpm", bufs=2,
                                       name=f"smpm_{it}")
                    nc.sync.dma_start(
                        sm_pm[:].rearrange("(r b) (c) -> r (b c)", r=128), smc[:]
                    )
                    nc.sync.dma_start(
                        smy[:], sm_pm[:].rearrange("(y f) c -> y (f c)", y=H)
                    )
                return smB, smy

            # it0 head first: its ACT exp beats the 98 setup exps into the
            # strict-FIFO scalar queue, so iteration 0 starts right after
            # the d2 matmuls finish.
            head0 = head(0)

            # ---------------- setup: E_b = exp(d2) in fp8 ----------------
            vb7_sb = ipool.tile([7, COLS], F32R, tag="qT_qfin", name="vb7")
            nc.sync.dma_start(vb7_sb[:], vb7[:])
            for b in range(NB):
                if b % 2 == 0:
                    ubc = stpool.tile([7, 256], F32R, tag="ubc", bufs=2,
                                      name=f"ubc_{b}")
                    nc.sync.dma_start(
                        ubc[:, 0:min(256, (NB - b) * 128)],
                        ub7[:, b * 128: min(N, (b + 2) * 128)],
                    )
                ps = pspool.tile([128, 2048], F32, tag=("psA" if b % 2 == 0 else "psB"),
                                 name=f"d2_{b}")
                for ci, (c0, cw) in enumerate(CTS):
                    mm(ps[:, ci * 512: ci * 512 + cw],
                       ubc[:, (b % 2) * 128: (b % 2 + 1) * 128],
                       vb7_sb[:, c0: c0 + cw], start=True, stop=True)
                nc.scalar.activation(E_res[:, b, :], ps[:, 0:COLS], EXPF)

            # ---------------- iterations ----------------
            for it in range(NITER):
                smB, smy = head0 if it == 0 else head(it)

                psB = pspool.tile([128, 2048], F32, tag="psB", name=f"psB_{it}")
                bl_ps = pspool.tile([32, 2048], F32, tag="psA", name=f"bl_{it}")

                def bl_chunk(b0, b1):
                    for b in range(b0, b1, 2):
                        for ci, (c0, cw) in enumerate(CTS):
                            mm(bl_ps[:, ci * 512: ci * 512 + cw],
                               smB[:, b: b + 2, :], E_res[:, b: b + 2, c0: c0 + cw],
                               start=(b == 0), stop=(b == NB - 2),
                               perf_mode=mybir.MatmulPerfMode.DoubleRow)

                bl_chunk(0, 56)

                # y-pass: Tk[k, (x c)] = gy2n^T @ smy  (y'-normalizer folded)
                Tk = ipool.tile([YPC, W * C], BF16, tag="Tk", name=f"Tk_{it}")
                XCH = [(0, 24), (24, 24), (48, 24), (72, 24), (96, 16)]
                for k, (x0, xw) in enumerate(XCH):
                    mm(psB[0:YPC, (k % 2) * 512: (k % 2) * 512 + xw * C],
                       gy2n_sb[:], smy[:, x0 * C: (x0 + xw) * C],
                       start=True, stop=True)
                    nc.vector.tensor_copy(
                        Tk[:, x0 * C: (x0 + xw) * C],
                        psB[0:YPC, (k % 2) * 512: (k % 2) * 512 + xw * C],
                    )
                # bounce to x-on-partitions (33K elems only)
                td = dpool.tile([YPC, W * C], BF16, tag="td", bufs=2, name=f"td_{it}")
                nc.sync.dma_start(td[:], Tk[:])
                Txk = ipool.tile([W, YPC * C], BF16, tag="Txk", name=f"Txk_{it}")
                nc.sync.dma_start(
                    Txk[:].rearrange("x (k c) -> x k c", k=YPC),
                    td[:].rearrange("k (x c) -> x k c", x=W),
                )

                bl_chunk(56, 80)

                # x-pass in 4 chunks of 28 output columns; each [28, (k c)]
                # PSUM block is padded to [32, (k, 32)] and DVE-32-transposed
                # into class partitions, then copied into the stacked rhs.
                so54 = ipool.tile([54, COLS], F32, tag="eqB_so54",
                                  name=f"so54_{it}")
                for j, x0 in enumerate([0, 28, 56, 84]):
                    pslab = psB[0:28, 1024 + (j % 2) * 512:
                                 1024 + (j % 2) * 512 + YPC * C]
                    mm(pslab, gxn_sb[:, x0: x0 + 28], Txk[:],
                       start=True, stop=True)
                    spc = ipool.tile([32, YPC * 32], BF16, tag="qy_spc",
                                     name=f"spc_{it}_{j}")
                    nc.gpsimd.memset(spc[:], 0.0)
                    nc.vector.tensor_copy(
                        spc[:].rearrange("p (k c) -> p k c", c=32)[0:28, :, 0:C],
                        pslab.rearrange("p (k c) -> p k c", c=C),
                    )
                    spT = ipool.tile([32, YPC * 32], BF16, tag="eqy_spT",
                                     name=f"spT_{it}_{j}")
                    nc.vector.transpose(spT[:], spc[:])
                    nc.vector.tensor_copy(
                        so54[32: 32 + C, :]
                        .rearrange("c (k x) -> c k x", k=YPC)[:, :, x0: x0 + 28],
                        spT[:].rearrange("p (k x) -> p k x", x=32)[0:C, :, 0:28],
                    )

                bl_chunk(80, NB)

                # ---- iteration 0: invnb = 1/nb broadcast across class rows
                if it == 0:
                    rnb32 = ipool.tile([32, COLS], F32, tag="qT_qfin", name="rnb32")
                    nc.vector.reciprocal(rnb32[:], bl_ps[:, 0:COLS])
                    rnb_d = dpool.tile([1, COLS], F32, tag="rnbd", name="rnbd")
                    nc.sync.dma_start(rnb_d[:], rnb32[C: C + 1, :])
                    nc.sync.dma_start(rnb32[0:1, :], rnb_d[:])
                    for ci, (c0, cw) in enumerate(CTS):
                        mm(psB[0:C, ci * 512: ci * 512 + cw],
                           ones21[:], rnb32[0:1, c0: c0 + cw], start=True, stop=True)
                    nc.vector.tensor_copy(invnb[:], psB[0:C, 0:COLS])

                # ---- stacked rhs rows 0:21 bilateral (rows 21:32 zero)
                nc.gpsimd.memset(so54[0:32, :], 0.0)
                nc.vector.tensor_mul(so54[0:C, :], bl_ps[0:C, 0:COLS], invnb[:])

                # ---- Q = u + [A_b ; A_s]^T @ so54
                q_ps = pspool.tile([C, 2048], F32, tag="psA", name=f"qps_{it}")
                for ci, (c0, cw) in enumerate(CTS):
                    mm(q_ps[:, ci * 512: ci * 512 + cw],
                       awT_sb[:], so54[:, c0: c0 + cw], start=True, stop=True)

                if it < NITER - 1:
                    nc.vector.tensor_add(q32[0:C, :], q_ps[:, 0:COLS], u21_sb[:])
                    qT = ipool.tile([32, COLS], BF16, tag="qT_qfin", name=f"qT_{it}")
                    nc.vector.transpose(qT[:], q32[:])
                    qt_sl = dpool.tile([COLS, C], BF16, tag="qtsl", bufs=2,
                                       name=f"qtsl_{it}")
                    nc.sync.dma_start(
                        qt_sl[:].rearrange("(k p) c -> p k c", p=32),
                        qT[:].rearrange("p (k c) -> p k c", c=32)[:, :, 0:C],
                    )
                    qt_full = dpool.tile([N, C], BF16, tag="qtfull", bufs=2,
                                         addr_space="Shared", name=f"qtfull_{it}")
                    nc.gpsimd.collective_compute(
                        "AllGather",
                        mybir.AluOpType.bypass,
                        replica_groups=[list(range(NCORES))],
                        ins=[qt_sl[:]],
                        outs=[qt_full[:]],
                    )
                else:
                    q_fin = ipool.tile([C, COLS], F32, tag="qT_qfin", name="qfin")
                    nc.vector.tensor_add(q_fin[:], q_ps[:, 0:COLS], u21_sb[:])
                    nc.sync.dma_start(qt_out[:], q_fin[:])

    nc.compile()
    return nc


def _host_inputs(unaries, rgb, spatial_kernel, bilateral_kernel, compatibility_matrix):
    bf = ml_dtypes.bfloat16
    u = np.transpose(np.asarray(unaries, dtype=np.float32)[0], (2, 0, 1)).reshape(C, N)
    rgbf = np.asarray(rgb, dtype=np.float32)[0].reshape(N, 3)

    yy, xx = np.meshgrid(
        np.arange(H, dtype=np.float64), np.arange(W, dtype=np.float64), indexing="ij"
    )
    pos = np.stack([xx.ravel(), yy.ravel()], axis=1)  # [N, 2] (x, y)

    fb = np.concatenate(
        [pos / THETA_ALPHA, rgbf.astype(np.float64) / THETA_BETA], axis=1
    )
    fb -= fb.mean(axis=0, keepdims=True)  # centering: reduces fp32 cancellation
    fb32 = fb.astype(np.float32)
    sq = (fb32.astype(np.float64) ** 2).sum(axis=1)
    mhalf_sq = (-0.5 * sq).astype(np.float32)

    ub7_np = np.empty((7, N), np.float32)
    ub7_np[0:5] = fb32.T
    ub7_np[5] = mhalf_sq
    ub7_np[6] = 1.0
    vb7_np = np.empty((7, N), np.float32)
    vb7_np[0:5] = fb32.T
    vb7_np[5] = 1.0
    vb7_np[6] = mhalf_sq

    d = np.arange(-(H - 1), H, dtype=np.float64)
    g1tab = np.exp(-(d * d) / (2.0 * THETA_GAMMA**2))

    def g1(dd):
        return g1tab[np.asarray(dd) + (H - 1)]

    G = g1(np.arange(W)[:, None] - np.arange(W)[None, :])  # [t, t']
    s1 = np.array([g1(np.arange(H) - t).sum() for t in range(H)])
    gxn_np = np.ascontiguousarray((G / s1[None, :]).astype(bf))  # [x, x']

    comp = np.asarray(compatibility_matrix, dtype=np.float64)
    A_s = -(comp @ np.asarray(spatial_kernel, dtype=np.float64))
    A_b = -(comp @ np.asarray(bilateral_kernel, dtype=np.float64))
    awT_np = np.zeros((54, C), np.float32)
    awT_np[0:C] = A_b.T.astype(np.float32)
    awT_np[32: 32 + C] = A_s.T.astype(np.float32)

    uT = np.ascontiguousarray(u.T)  # [N, C]
    # contraction block b holds pixels {p*98+b}; DMA column (b,p) = pixel 98p+b
    X = (98 * np.arange(128)[None, :] + np.arange(NB)[:, None]).reshape(-1)
    ub7_np = np.ascontiguousarray(ub7_np[:, X])
    qblk0_np = np.ascontiguousarray(
        uT.reshape(128, NB, C).reshape(128, NB * C).astype(bf)
    )
    qy0_np = np.ascontiguousarray(uT.reshape(H, W * C).astype(bf))

    in_maps = []
    for c in range(NCORES):
        sl = slice(c * COLS, (c + 1) * COLS)
        dy = np.arange(H)[:, None] - (YPC * c + np.arange(YPC))[None, :]  # [y, k]
        gy2n_np = np.ascontiguousarray(
            (g1(dy) / s1[YPC * c + np.arange(YPC)][None, :]).astype(bf)
        )
        in_maps.append(
            dict(
                ub7=ub7_np,
                vb7=np.ascontiguousarray(vb7_np[:, sl]),
                gxn=gxn_np,
                gy2n=gy2n_np,
                u21=np.ascontiguousarray(u[:, sl]),
                qblk0=qblk0_np,
                qy0=qy0_np,
                awT=awT_np,
            )
        )
    return in_maps


def run(inputs, trace=False, **spmd_kwargs):
    in_maps = _host_inputs(**inputs)
    if "nc" not in _CACHE:
        _CACHE["nc"] = _build_program()
    nc = _CACHE["nc"]
    res = run_bass_kernel_spmd(
        nc, in_maps, core_ids=list(range(NCORES)), trace=trace, **spmd_kwargs
    )
    qs = [np.asarray(res.results[c]["qt_out"]) for c in range(NCORES)]
    Q = np.concatenate(qs, axis=1)  # [C, N]
    out = Q.reshape(C, H, W).transpose(1, 2, 0)[None].astype(np.float32)
    return out, res


def kernel(unaries, rgb, spatial_kernel, bilateral_kernel, compatibility_matrix):
    out, _ = run(
        dict(
            unaries=unaries,
            rgb=rgb,
            spatial_kernel=spatial_kernel,
            bilateral_kernel=bilateral_kernel,
            compatibility_matrix=compatibility_matrix,
        )
    )
    return out
